# revision 3
# baseline (speedup 1.0000x reference)
"""
Trainium2 Bass kernel for nn_LinearLUT (residual-binarized LUT linear layer).

Math restructure (v2)
---------------------
out[b,o] = bias[o] + sum_l sum_j Q_l[t, code_l[b,t]],  t=(o,j),
code_l[b,t] = sum_i 2^i * bit_l[b, idx_i(t)]  (4-bit code, ONE matmul/tile).

Split each 16-entry LUT row Q_l[t,:] into a bit-AFFINE part + residual:
  alpha = Q[0],  gamma_i = Q[2^i] - Q[0]
  R[t,v] = Q[t,v] - alpha - sum_i gamma_i*bit_i(v)   (zero at v in {0,1,2,4,8})
The affine part needs NO one-hot planes -- it contracts to a single dense
matmul per level with a static matrix Gaff[j',o] = sum_{j,i}[idx_i(o,j)=j']
gamma_i[o,j]; only the 11 residual codes need one-hot planes.

Plane production (DVE): codes of BOTH levels live in one [128, 4096] fp16
tensor -> one is_equal per v covers both levels (11 ops, 4x DVE mode).

Contraction (PE, no per-plane LDWEIGHTS): for each (l,v) the stationary is
the tiny residual slice Qres[j, o16]; the eq plane streams as the moving
operand producing psum[o', (o,b)] -- only the block diagonal o'==o is
meaningful.  cvec (rank-1) and the affine matmuls stream diag-compatibly
into the same [16, 2048] psum (all rows of a streamed column get the same
row-o value, so the diagonal picks the right one).  Host extracts
res[b, o] = y[o, o*128+b] -- pure indexing.
"""

import numpy as np

import concourse.bass as bass
import concourse.bacc as bacc
import concourse.mybir as mybir
import concourse.tile as tile
from concourse.bass_utils import run_bass_kernel_spmd

# Problem dims (hardcoded per contract)
LEVELS = 2
K = 4
KK = 16
IN = 128
OUT = 128
B = 128
T = IN * OUT  # 16384
NCORES = 8
T_C = T // NCORES     # 2048 tables per core
OL = OUT // NCORES    # 16 out features per core
NTILE = T_C // 128    # 16 t-tiles per core

# residual codes: everything except the affine-absorbed {0,1,2,4,8}
VRES = [3, 5, 6, 7, 9, 10, 11, 12, 13, 14, 15]
NV = len(VRES)  # 11

F16 = mybir.dt.float16
F32 = mybir.dt.float32

_CACHED_NC = None


def _build_nc():
    """Build the per-core Bass program (identical on all 8 cores)."""
    nc = bacc.Bacc("TRN2", target_bir_lowering=False, debug=False,
                   num_devices=NCORES)

    xt = nc.dram_tensor("xt", [IN, B], F32, kind="ExternalInput")
    consts = nc.dram_tensor("consts", [128, 2], F32, kind="ExternalInput")
    g = nc.dram_tensor("g", [IN, T_C], F16, kind="ExternalInput")
    gaff = nc.dram_tensor("gaff", [IN, LEVELS * OL], F16, kind="ExternalInput")
    qres = nc.dram_tensor("qres", [128, LEVELS * NV * OL], F16,
                          kind="ExternalInput")
    cvec = nc.dram_tensor("cvec", [1, OL], F16, kind="ExternalInput")
    y = nc.dram_tensor("y", [OL, NTILE * B], F16, kind="ExternalOutput")

    NFILL = 8            # code-psum fills (2 tiles x 2 levels each)
    NCHUNK = 4           # 512-col chunks of the [16, 2048] output psum

    with tile.TileContext(nc) as tc:
        with (
            tc.tile_pool(name="const", bufs=1) as cpool,
            tc.tile_pool(name="bits", bufs=1) as bpool,
            tc.tile_pool(name="codes", bufs=1) as cdpool,
            tc.tile_pool(name="eq", bufs=4) as eqpool,
            tc.tile_pool(name="out", bufs=1) as opool,
            tc.tile_pool(name="psum_code", bufs=4,
                         space=bass.MemorySpace.PSUM) as pc,
            tc.tile_pool(name="psum_y", bufs=4,
                         space=bass.MemorySpace.PSUM) as py,
        ):
            xt_sb = cpool.tile([IN, B], F32, tag="xt")
            c_sb = cpool.tile([128, 2], F32, tag="consts")
            cv_sb = cpool.tile([1, OL], F16, tag="cvec")
            ones_sb = cpool.tile([1, 512], F16, tag="ones")
            gaff_sb = cpool.tile([IN, LEVELS * OL], F16, tag="gaff")
            qres_sb = cpool.tile([128, LEVELS * NV * OL], F16, tag="qres")
            nc.sync.dma_start(xt_sb[:], xt[:])
            nc.sync.dma_start(c_sb[:], consts[:])
            nc.sync.dma_start(cv_sb[:], cvec[:])
            nc.sync.dma_start(gaff_sb[:], gaff[:])
            nc.sync.dma_start(qres_sb[:], qres[:])
            nc.gpsimd.memset(ones_sb[:], 1.0)
            # g in 4 chunks so code matmuls start before the full DMA lands
            g_sb = []
            for q in range(4):
                gq = cpool.tile([IN, 4 * 128], F16, tag=f"g{q}",
                                name=f"g_sb{q}")
                nc.sync.dma_start(gq[:], g[:, q * 512:(q + 1) * 512])
                g_sb.append(gq)

            # ---- sign bits (as fp16 0/1, j on partitions) ----
            bit1 = bpool.tile([IN, B], F16, tag="bit1")
            nc.vector.tensor_scalar(bit1[:], xt_sb[:], 0.0, None,
                                    mybir.AluOpType.is_ge)
            # rc = x - 2*m0*bit1   (== resid - m0)
            rc = bpool.tile([IN, B], F32, tag="rc")
            nc.vector.scalar_tensor_tensor(rc[:], bit1[:], c_sb[:, 0:1],
                                           xt_sb[:], mybir.AluOpType.mult,
                                           mybir.AluOpType.add)
            # bit2 = (rc >= -m0)
            bit2 = bpool.tile([IN, B], F16, tag="bit2")
            nc.vector.tensor_scalar(bit2[:], rc[:], c_sb[:, 1:2], None,
                                    mybir.AluOpType.is_ge)
            bits = [bit1, bit2]

            # ---- output psum chunks; seed with cvec + affine ----
            y_ps = [py.tile([OL, 512], F32, tag="ypsum", name=f"yps{c}")
                    for c in range(NCHUNK)]
            for c in range(NCHUNK):
                # rank-1 seed: row o gets cvec[o] in every column
                nc.tensor.matmul(y_ps[c][:], cv_sb[:],
                                 ones_sb[:], start=True, stop=False)
            for l in range(LEVELS):
                for c in range(NCHUNK):
                    for qb in range(4):
                        nc.tensor.matmul(
                            y_ps[c][:, qb * 128:(qb + 1) * 128],
                            gaff_sb[:, l * OL:(l + 1) * OL],
                            bits[l][:],
                            start=False, stop=False,
                        )

            # ---- code matmuls + PSUM->SBUF drains ----
            # codes_cat[j, (l, tile, b)]: both levels in one fp16 tensor
            codes = cdpool.tile([128, LEVELS * NTILE * B], F16, tag="codes")
            for f in range(NFILL):
                cps = pc.tile([128, 512], F32, tag="codepsum",
                              name=f"cps{f}")
                for dt_i in range(2):
                    t_i = 2 * f + dt_i
                    gq = g_sb[t_i // 4]
                    gcol = (t_i % 4) * 128
                    for l in range(LEVELS):
                        nc.tensor.matmul(
                            cps[:, l * 256 + dt_i * 128:
                                l * 256 + dt_i * 128 + 128],
                            gq[:, gcol:gcol + 128],
                            bits[l][:],
                            start=True, stop=True,
                        )
                # one drain per fill: src (l, dt, b) -> dst cols
                # l*2048 + (2f)*128 .. +256
                src = cps[:].rearrange("p (l d b) -> p l (d b)", l=2, d=2)
                dst = codes[:].rearrange("p (l t b) -> p l (t b)", l=2, t=16)[
                    :, :, 2 * f * 128:2 * f * 128 + 256]
                nc.scalar.copy(dst, src)

            # ---- one-hot residual planes + streamed contraction ----
            n_steps = NV * LEVELS
            for vi, v in enumerate(VRES):
                eq = eqpool.tile([128, LEVELS * NTILE * B], F16, tag="eq")
                nc.vector.tensor_scalar(eq[:], codes[:], float(v), None,
                                        mybir.AluOpType.is_equal)
                for l in range(LEVELS):
                    step = vi * LEVELS + l
                    qcol = (l * NV + vi) * OL
                    for c in range(NCHUNK):
                        nc.tensor.matmul(
                            y_ps[c][:],
                            qres_sb[:, qcol:qcol + OL],
                            eq[:, l * 2048 + c * 512:l * 2048 + c * 512 + 512],
                            start=False,
                            stop=(step == n_steps - 1),
                        )

            y_sb = opool.tile([OL, NTILE * B], F16, tag="ysb")
            for c in range(NCHUNK):
                nc.scalar.copy(y_sb[:, c * 512:(c + 1) * 512], y_ps[c][:])
            nc.sync.dma_start(y[:], y_sb[:])

    nc.compile()
    return nc


def _host_prep(x, weight, bias, means):
    """Weight-static preprocessing: per-level LUTs Q_l[t, v] (fp64)."""
    w = weight.astype(np.float64)
    m = np.abs(means.astype(np.float64))
    cc = np.arange(KK)
    tt = (2 * ((cc[:, None] >> np.arange(K)[None, :]) & 1) - 1).astype(
        np.float64)          # [c, i]
    sig = tt                  # same construction for sign patterns [v, i]

    qs = []
    for l in range(LEVELS):
        # M[v, c] = prod_i (1 + m_l * sig[v,i] * tt[c,i]) / 2
        M = np.prod((1.0 + m[l] * sig[:, None, :] * tt[None, :, :]) * 0.5,
                    axis=-1)  # [v, c]
        q = w @ M.T           # [T, KK]
        qs.append(q)
    return qs


def _affine_split(qs):
    """Split each LUT row into bit-affine part + residual.

    Returns alphas[l][T], gammas[l][T, 4], resid[l][T, NV]."""
    vbits = ((np.array(VRES)[:, None] >> np.arange(K)[None, :]) & 1).astype(
        np.float64)  # [NV, 4]
    alphas, gammas, resid = [], [], []
    for l in range(LEVELS):
        q = qs[l]
        a = q[:, 0]
        g = np.stack([q[:, 1 << i] - a for i in range(K)], axis=1)  # [T, 4]
        r = q[:, VRES] - a[:, None] - g @ vbits.T                   # [T, NV]
        alphas.append(a)
        gammas.append(g)
        resid.append(r)
    return alphas, gammas, resid


def _build_g(input_mask):
    G = np.zeros((IN, T), np.float64)
    cols = np.repeat(np.arange(T), K)
    vals = np.tile(2.0 ** np.arange(K), T)
    np.add.at(G, (input_mask.astype(np.int64), cols), vals)
    return G


def _build_gaff(input_mask, gammas):
    """Gaff_l[j', t] = sum_i [idx_i(t)==j'] * gamma_i[l, t]  (full T)."""
    idx = input_mask.astype(np.int64).reshape(T, K)
    gaffs = []
    for l in range(LEVELS):
        Ga = np.zeros((IN, T), np.float64)
        for i in range(K):
            np.add.at(Ga, (idx[:, i], np.arange(T)), gammas[l][:, i])
        gaffs.append(Ga)
    return gaffs


def _make_in_maps(x, weight, bias, means, input_mask):
    qs = _host_prep(x, weight, bias, means)
    alphas, gammas, resid = _affine_split(qs)
    G = _build_g(input_mask)
    gaffs = _build_gaff(input_mask, gammas)

    m0 = float(np.abs(means.astype(np.float64))[0])
    consts = np.zeros((128, 2), np.float32)
    consts[:, 0] = -2.0 * m0
    consts[:, 1] = -m0
    xt = np.ascontiguousarray(x.astype(np.float32).T)

    # cvec[o] = bias[o] + sum_l sum_j alpha_l[o*IN+j]
    cvec_full = bias.astype(np.float64).copy()
    for l in range(LEVELS):
        cvec_full += alphas[l].reshape(OUT, IN).sum(-1)

    in_maps = []
    for cid in range(NCORES):
        t0 = cid * T_C
        gc = G[:, t0:t0 + T_C].astype(np.float16)
        # gaff[j', (l, o)]: sum over this core's o-range tables of gamma
        gaff_c = np.empty((IN, LEVELS, OL), np.float64)
        # qres[j, (l, vi, o)] = resid_l[(o*IN+j), vi]
        qres_c = np.empty((128, LEVELS, NV, OL), np.float64)
        for l in range(LEVELS):
            Ga = gaffs[l][:, t0:t0 + T_C].reshape(IN, OL, IN)
            gaff_c[:, l, :] = Ga.sum(-1)
            rc = resid[l][t0:t0 + T_C].reshape(OL, IN, NV)
            qres_c[:, l, :, :] = rc.transpose(1, 2, 0)
        in_maps.append({
            "xt": xt,
            "consts": consts,
            "g": np.ascontiguousarray(gc),
            "gaff": np.ascontiguousarray(
                gaff_c.reshape(IN, -1).astype(np.float16)),
            "qres": np.ascontiguousarray(
                qres_c.reshape(128, -1).astype(np.float16)),
            "cvec": np.ascontiguousarray(
                cvec_full[cid * OL:(cid + 1) * OL].astype(
                    np.float16)[None, :]),
        })
    return in_maps


def kernel(x, weight, bias, means, input_mask):
    global _CACHED_NC
    if _CACHED_NC is None:
        _CACHED_NC = _build_nc()
    nc = _CACHED_NC

    in_maps = _make_in_maps(x, weight, bias, means, input_mask)
    res = run_bass_kernel_spmd(nc, in_maps, list(range(NCORES)))
    globals()["_LAST_RESULTS"] = res
    # y[o_local, tile*128 + b]: diagonal tile == o_local holds the result
    out = np.empty((B, OUT), np.float32)
    for cid in range(NCORES):
        yc = res.results[cid]["y"].astype(np.float32)  # [OL, 2048]
        for o in range(OL):
            out[:, cid * OL + o] = yc[o, o * 128:(o + 1) * 128]
    return out


# revision 6
# speedup vs baseline: 1.2502x; 1.2502x over previous
"""
Trainium2 Bass kernel for nn_LinearLUT (residual-binarized LUT linear layer).

Math restructure (v3)
---------------------
out[b,o] = bias[o] + sum_l sum_j Q_l[t, code_l[b,t]],  t=(o,j),
code_l[b,t] = sum_i 2^i * bit_l[b, idx_i(t)]  (4-bit code, ONE matmul/tile).

Fit each 16-entry LUT row Q_l[t,:] with an 11-parameter model in the code
bits: alpha + sum_i gamma_i b_i + sum_{i<k} c_ik b_i b_k, matched EXACTLY
at the 11 codes with <=2 bits set.  The residual R is nonzero only at the
5 codes {7,11,13,14,15}.

Device evaluation per level l:
  quadratic+linear: fold gamma onto the diagonal of a static per-(l,o)
    matrix M (bit^2 = bit), then
      Z_{l,o} = M_{l,o}^T @ bits_l          (PE, 32 matmuls)
      u = Z * bits_l                        (DVE, from PSUM)
      y_quad[o',(o,b)] += ones^T u          (PE; every row gets the column
                                             sum -> diagonal picks row o)
  constant:  rank-1 cvec stream (diag-compatible)
  residual:  5 one-hot planes (is_equal over the level-concatenated code
    tensor) streamed against tiny stationary Qres[j, o16] into the same
    [16, 2048] psum -- only the block diagonal o'==o is meaningful.

Host extracts res[b, o] = y[o, o*128+b] -- pure indexing -- and
concatenates cores (o is sharded 16/core).
"""

import numpy as np

import concourse.bass as bass
import concourse.bacc as bacc
import concourse.mybir as mybir
import concourse.tile as tile
from concourse.bass_utils import run_bass_kernel_spmd

# Problem dims (hardcoded per contract)
LEVELS = 2
K = 4
KK = 16
IN = 128
OUT = 128
B = 128
T = IN * OUT  # 16384
NCORES = 8
T_C = T // NCORES     # 2048 tables per core
OL = OUT // NCORES    # 16 out features per core
NTILE = T_C // 128    # 16 t-tiles per core

# codes with <=2 bits set are absorbed by the quadratic fit
C11 = [0, 1, 2, 4, 8, 3, 5, 6, 9, 10, 12]
VRES = [7, 11, 13, 14, 15]   # >=3 bits set: residual one-hot planes
NV = len(VRES)  # 5
PAIRS = [(0, 1), (0, 2), (0, 3), (1, 2), (1, 3), (2, 3)]

F16 = mybir.dt.float16
F32 = mybir.dt.float32

_CACHED_NC = None


def _build_nc():
    """Build the per-core Bass program (identical on all 8 cores)."""
    nc = bacc.Bacc("TRN2", target_bir_lowering=False, debug=False,
                   num_devices=NCORES)

    xt = nc.dram_tensor("xt", [IN, B], F32, kind="ExternalInput")
    consts = nc.dram_tensor("consts", [128, 2], F32, kind="ExternalInput")
    g = nc.dram_tensor("g", [IN, T_C], F16, kind="ExternalInput")
    mq = nc.dram_tensor("mq", [128, LEVELS * OL * 128], F16,
                        kind="ExternalInput")
    qres = nc.dram_tensor("qres", [128, LEVELS * NV * OL], F16,
                          kind="ExternalInput")
    cvec = nc.dram_tensor("cvec", [1, OL], F16, kind="ExternalInput")
    y = nc.dram_tensor("y", [OL, NTILE * B], F16, kind="ExternalOutput")

    NFILL = 8            # code-psum fills (2 tiles x 2 levels each)
    NCHUNK = 4           # 512-col chunks of the [16, 2048] output psum
    NWAVE = 8            # Z waves (4 (l,o) pairs each)

    with tile.TileContext(nc) as tc:
        with (
            tc.tile_pool(name="const", bufs=1) as cpool,
            tc.tile_pool(name="bits", bufs=1) as bpool,
            tc.tile_pool(name="codes", bufs=1) as cdpool,
            tc.tile_pool(name="eq", bufs=3) as eqpool,
            tc.tile_pool(name="u", bufs=1) as upool,
            tc.tile_pool(name="out", bufs=1) as opool,
            tc.tile_pool(name="psum_code", bufs=2,
                         space=bass.MemorySpace.PSUM) as pc,
            tc.tile_pool(name="psum_z", bufs=2,
                         space=bass.MemorySpace.PSUM) as pz,
            tc.tile_pool(name="psum_y", bufs=4,
                         space=bass.MemorySpace.PSUM) as py,
        ):
            xt_sb = cpool.tile([IN, B], F32, tag="xt")
            c_sb = cpool.tile([128, 2], F32, tag="consts")
            cv_sb = cpool.tile([1, OL], F16, tag="cvec")
            ones_sb = cpool.tile([1, 512], F16, tag="ones")
            onesq_sb = cpool.tile([128, OL], F16, tag="onesq")
            qres_sb = cpool.tile([128, LEVELS * NV * OL], F16, tag="qres")
            nc.sync.dma_start(xt_sb[:], xt[:])
            nc.sync.dma_start(c_sb[:], consts[:])
            # g in 4 chunks so code matmuls start before the full DMA lands
            g_sb = []
            for q in range(4):
                gq = cpool.tile([IN, 4 * 128], F16, tag=f"g{q}",
                                name=f"g_sb{q}")
                nc.sync.dma_start(gq[:], g[:, q * 512:(q + 1) * 512])
                g_sb.append(gq)
            # M matrices [p, (l, o, q)] in 4 chunks
            mq_sb = cpool.tile([128, LEVELS * OL * 128], F16, tag="mq")
            MCH = LEVELS * OL * 128 // 4
            for q in range(4):
                nc.sync.dma_start(mq_sb[:, q * MCH:(q + 1) * MCH],
                                  mq[:, q * MCH:(q + 1) * MCH])
            nc.sync.dma_start(qres_sb[:], qres[:])
            nc.sync.dma_start(cv_sb[:], cvec[:])
            nc.gpsimd.memset(ones_sb[:], 1.0)
            nc.gpsimd.memset(onesq_sb[:], 1.0)

            # ---- sign bits (fp16 0/1, j on partitions), both levels in one
            # [128, 256] tensor so each code matmul streams 256 cols ----
            bits_cat = bpool.tile([IN, LEVELS * B], F16, tag="bits")
            bit1 = bits_cat[:, 0:B]
            bit2 = bits_cat[:, B:2 * B]
            nc.vector.tensor_scalar(bit1, xt_sb[:], 0.0, None,
                                    mybir.AluOpType.is_ge)
            rc = bpool.tile([IN, B], F32, tag="rc")
            nc.vector.scalar_tensor_tensor(rc[:], bit1, c_sb[:, 0:1],
                                           xt_sb[:], mybir.AluOpType.mult,
                                           mybir.AluOpType.add)
            nc.vector.tensor_scalar(bit2, rc[:], c_sb[:, 1:2], None,
                                    mybir.AluOpType.is_ge)
            # replicated bits for the qmult waves (4 o's per wave)
            brep = bpool.tile([IN, LEVELS * 512], F16, tag="brep")
            for l in range(LEVELS):
                for r in range(4):
                    nc.vector.tensor_copy(
                        brep[:, l * 512 + r * 128:l * 512 + r * 128 + 128],
                        bits_cat[:, l * B:(l + 1) * B])

            # ---- code matmuls + PSUM->SBUF drains ----
            # codes_cat[j, (l, tile, b)]: both levels in one fp16 tensor
            codes = cdpool.tile([128, LEVELS * NTILE * B], F16, tag="codes")
            for f in range(NFILL):
                cps = pc.tile([128, 512], F32, tag="codepsum",
                              name=f"cps{f}")
                for dt_i in range(2):
                    t_i = 2 * f + dt_i
                    gq = g_sb[t_i // 4]
                    gcol = (t_i % 4) * 128
                    # one matmul streams both levels (256 cols)
                    nc.tensor.matmul(
                        cps[:].rearrange("p (d l b) -> p d (l b)", d=2,
                                         l=2)[:, dt_i],
                        gq[:, gcol:gcol + 128],
                        bits_cat[:],
                        start=True, stop=True,
                    )
                # drain: src (d, l, b) -> codes[(l, t, b)] at tiles 2f, 2f+1
                src = cps[:].rearrange("p (d l b) -> p l d b", d=2, l=2)
                dst = codes[:].rearrange("p (l t b) -> p l t b", l=2,
                                         t=16)[:, :, 2 * f:2 * f + 2, :]
                nc.scalar.copy(dst, src)

            # ---- output psum chunks; seed with cvec (start=True) ----
            y_ps = [py.tile([OL, 512], F32, tag="ypsum", name=f"yps{c}")
                    for c in range(NCHUNK)]
            for c in range(NCHUNK):
                nc.tensor.matmul(y_ps[c][:], cv_sb[:],
                                 ones_sb[:], start=True, stop=False)

            # ---- quadratic forms: Z = M^T @ bits per (l, o) ----
            zps = []
            for w in range(NWAVE):
                zw = pz.tile([128, 512], F32, tag="zpsum", name=f"z{w}")
                zps.append(zw)
                l = w // 4
                for zo in range(4):
                    o = (w % 4) * 4 + zo
                    mcol = (l * OL + o) * 128
                    nc.tensor.matmul(
                        zw[:, zo * 128:(zo + 1) * 128],
                        mq_sb[:, mcol:mcol + 128],
                        bits_cat[:, l * B:(l + 1) * B],
                        start=True, stop=True,
                    )
            # u = Z * bits  (DVE reads PSUM, writes fp16 SBUF)
            u_sb = upool.tile([128, LEVELS * OL * B], F16, tag="u")
            for w in range(NWAVE):
                l = w // 4
                nc.vector.tensor_tensor(
                    u_sb[:, w * 512:(w + 1) * 512],
                    zps[w][:],
                    brep[:, l * 512:(l + 1) * 512],
                    mybir.AluOpType.mult)

            # ---- one-hot residual planes + streamed contraction ----
            n_steps = NV * LEVELS + LEVELS  # LUT matmuls + u-sum matmuls
            for vi, v in enumerate(VRES):
                eq = eqpool.tile([128, LEVELS * NTILE * B], F16, tag="eq")
                nc.vector.tensor_scalar(eq[:], codes[:], float(v), None,
                                        mybir.AluOpType.is_equal)
                for l in range(LEVELS):
                    qcol = (l * NV + vi) * OL
                    for c in range(NCHUNK):
                        nc.tensor.matmul(
                            y_ps[c][:],
                            qres_sb[:, qcol:qcol + OL],
                            eq[:, l * 2048 + c * 512:l * 2048 + c * 512 + 512],
                            start=False, stop=False,
                        )

            # ---- u-sum: every output row gets the column sum of u ----
            for l in range(LEVELS):
                for c in range(NCHUNK):
                    nc.tensor.matmul(
                        y_ps[c][:],
                        onesq_sb[:],
                        u_sb[:, l * 2048 + c * 512:l * 2048 + c * 512 + 512],
                        start=False, stop=(l == LEVELS - 1),
                    )

            y_sb = opool.tile([OL, NTILE * B], F16, tag="ysb")
            for c in range(NCHUNK):
                nc.scalar.copy(y_sb[:, c * 512:(c + 1) * 512], y_ps[c][:])
            nc.sync.dma_start(y[:], y_sb[:])

    nc.compile()
    return nc


def _host_prep(x, weight, bias, means):
    """Weight-static preprocessing: per-level LUTs Q_l[t, v] (fp64)."""
    w = weight.astype(np.float64)
    m = np.abs(means.astype(np.float64))
    cc = np.arange(KK)
    tt = (2 * ((cc[:, None] >> np.arange(K)[None, :]) & 1) - 1).astype(
        np.float64)          # [c, i]
    sig = tt                  # same construction for sign patterns [v, i]

    qs = []
    for l in range(LEVELS):
        # M[v, c] = prod_i (1 + m_l * sig[v,i] * tt[c,i]) / 2
        M = np.prod((1.0 + m[l] * sig[:, None, :] * tt[None, :, :]) * 0.5,
                    axis=-1)  # [v, c]
        q = w @ M.T           # [T, KK]
        qs.append(q)
    return qs


def _feat(codes):
    """11-dim feature vector [1, b0..b3, pair products] per code."""
    codes = np.asarray(codes)
    b = ((codes[:, None] >> np.arange(K)[None, :]) & 1).astype(np.float64)
    cols = [np.ones(len(codes))] + [b[:, i] for i in range(K)]
    cols += [b[:, i] * b[:, k] for (i, k) in PAIRS]
    return np.stack(cols, axis=1)  # [n, 11]


def _quad_fit(qs):
    """Fit alpha/gamma/pair coefs exactly at C11; residual at VRES.

    Returns coef[l][T, 11] and resid[l][T, NV]."""
    A = _feat(C11)                 # [11, 11]
    Ainv = np.linalg.inv(A)
    Fres = _feat(VRES)             # [NV, 11]
    coefs, resid = [], []
    for l in range(LEVELS):
        c = qs[l][:, C11] @ Ainv.T          # [T, 11]
        r = qs[l][:, VRES] - c @ Fres.T     # [T, NV]
        coefs.append(c)
        resid.append(r)
    return coefs, resid


def _build_g(input_mask):
    G = np.zeros((IN, T), np.float64)
    cols = np.repeat(np.arange(T), K)
    vals = np.tile(2.0 ** np.arange(K), T)
    np.add.at(G, (input_mask.astype(np.int64), cols), vals)
    return G


def _build_m(input_mask, coefs, t0):
    """M[p, (l, o, q)] for this core's OL out-features starting at table t0.

    Quadratic-form matrices: diagonal gets gamma_i at idx_i; entry
    (idx_i, idx_k) accumulates the pair coefficient (single-sided; the
    device computes bits^T M bits so diagonal collisions are absorbed by
    bit^2 = bit)."""
    idx = input_mask.astype(np.int64).reshape(T, K)[t0:t0 + T_C]  # [T_C, 4]
    Ms = np.zeros((LEVELS, OL, 128, 128), np.float64)
    tloc = np.arange(T_C)
    o_of_t = tloc // IN
    for l in range(LEVELS):
        cf = coefs[l][t0:t0 + T_C]  # [T_C, 11]
        for i in range(K):
            np.add.at(Ms, (l, o_of_t, idx[:, i], idx[:, i]), cf[:, 1 + i])
        for pi, (i, k) in enumerate(PAIRS):
            np.add.at(Ms, (l, o_of_t, idx[:, i], idx[:, k]), cf[:, 5 + pi])
    # -> [p, (l, o, q)]
    return Ms.transpose(2, 0, 1, 3).reshape(128, -1)


def _make_in_maps(x, weight, bias, means, input_mask):
    qs = _host_prep(x, weight, bias, means)
    coefs, resid = _quad_fit(qs)
    G = _build_g(input_mask)

    m0 = float(np.abs(means.astype(np.float64))[0])
    consts = np.zeros((128, 2), np.float32)
    consts[:, 0] = -2.0 * m0
    consts[:, 1] = -m0
    xt = np.ascontiguousarray(x.astype(np.float32).T)

    # cvec[o] = bias[o] + sum_l sum_j alpha_l[o*IN+j]
    cvec_full = bias.astype(np.float64).copy()
    for l in range(LEVELS):
        cvec_full += coefs[l][:, 0].reshape(OUT, IN).sum(-1)

    in_maps = []
    for cid in range(NCORES):
        t0 = cid * T_C
        gc = G[:, t0:t0 + T_C].astype(np.float16)
        mc = _build_m(input_mask, coefs, t0).astype(np.float16)
        # qres[j, (l, vi, o)] = resid_l[(o*IN+j), vi]
        qres_c = np.empty((128, LEVELS, NV, OL), np.float64)
        for l in range(LEVELS):
            rc = resid[l][t0:t0 + T_C].reshape(OL, IN, NV)
            qres_c[:, l, :, :] = rc.transpose(1, 2, 0)
        in_maps.append({
            "xt": xt,
            "consts": consts,
            "g": np.ascontiguousarray(gc),
            "mq": np.ascontiguousarray(mc),
            "qres": np.ascontiguousarray(
                qres_c.reshape(128, -1).astype(np.float16)),
            "cvec": np.ascontiguousarray(
                cvec_full[cid * OL:(cid + 1) * OL].astype(
                    np.float16)[None, :]),
        })
    return in_maps


def kernel(x, weight, bias, means, input_mask):
    global _CACHED_NC
    if _CACHED_NC is None:
        _CACHED_NC = _build_nc()
    nc = _CACHED_NC

    in_maps = _make_in_maps(x, weight, bias, means, input_mask)
    res = run_bass_kernel_spmd(nc, in_maps, list(range(NCORES)))
    globals()["_LAST_RESULTS"] = res
    # y[o_local, tile*128 + b]: diagonal tile == o_local holds the result
    out = np.empty((B, OUT), np.float32)
    for cid in range(NCORES):
        yc = res.results[cid]["y"].astype(np.float32)  # [OL, 2048]
        for o in range(OL):
            out[:, cid * OL + o] = yc[o, o * 128:(o + 1) * 128]
    return out


# revision 12
# speedup vs baseline: 1.3039x; 1.0430x over previous
"""
Trainium2 Bass kernel for nn_LinearLUT (residual-binarized LUT linear layer).

Math restructure (v3)
---------------------
out[b,o] = bias[o] + sum_l sum_j Q_l[t, code_l[b,t]],  t=(o,j),
code_l[b,t] = sum_i 2^i * bit_l[b, idx_i(t)]  (4-bit code, ONE matmul/tile).

Fit each 16-entry LUT row Q_l[t,:] with an 11-parameter model in the code
bits: alpha + sum_i gamma_i b_i + sum_{i<k} c_ik b_i b_k, matched EXACTLY
at the 11 codes with <=2 bits set.  The residual R is nonzero only at the
5 codes {7,11,13,14,15}.

Device evaluation per level l:
  quadratic+linear: fold gamma onto the diagonal of a static per-(l,o)
    matrix M (bit^2 = bit), then
      Z_{l,o} = M_{l,o}^T @ bits_l          (PE, 32 matmuls)
      u = Z * bits_l                        (DVE, from PSUM)
      y_quad[o',(o,b)] += ones^T u          (PE; every row gets the column
                                             sum -> diagonal picks row o)
  constant:  rank-1 cvec stream (diag-compatible)
  residual:  5 one-hot planes (is_equal over the level-concatenated code
    tensor) streamed against tiny stationary Qres[j, o16] into the same
    [16, 2048] psum -- only the block diagonal o'==o is meaningful.

Host extracts res[b, o] = y[o, o*128+b] -- pure indexing -- and
concatenates cores (o is sharded 16/core).
"""

import numpy as np

import concourse.bass as bass
import concourse.bacc as bacc
import concourse.mybir as mybir
import concourse.tile as tile
from concourse.bass_utils import run_bass_kernel_spmd

# Problem dims (hardcoded per contract)
LEVELS = 2
K = 4
KK = 16
IN = 128
OUT = 128
B = 128
T = IN * OUT  # 16384
NCORES = 8
T_C = T // NCORES     # 2048 tables per core
OL = OUT // NCORES    # 16 out features per core
NTILE = T_C // 128    # 16 t-tiles per core

# codes with <=2 bits set are absorbed by the quadratic fit
C11 = [0, 1, 2, 4, 8, 3, 5, 6, 9, 10, 12]
VRES = [7, 11, 13, 14, 15]   # >=3 bits set: residual one-hot planes
NV = len(VRES)  # 5
PAIRS = [(0, 1), (0, 2), (0, 3), (1, 2), (1, 3), (2, 3)]

F16 = mybir.dt.float16
F32 = mybir.dt.float32

_CACHED_NC = None


def _build_nc():
    """Build the per-core Bass program (identical on all 8 cores)."""
    nc = bacc.Bacc("TRN2", target_bir_lowering=False, debug=False,
                   num_devices=NCORES)

    xt = nc.dram_tensor("xt", [IN, B], F32, kind="ExternalInput")
    consts = nc.dram_tensor("consts", [128, 2], F32, kind="ExternalInput")
    g = nc.dram_tensor("g", [IN, T_C], F16, kind="ExternalInput")
    mq = nc.dram_tensor("mq", [128, LEVELS * OL * 128], F16,
                        kind="ExternalInput")
    qres = nc.dram_tensor("qres", [128, LEVELS * NV * OL], F16,
                          kind="ExternalInput")
    y = nc.dram_tensor("y", [OL, NTILE * B], F16, kind="ExternalOutput")

    NFILL = 8            # code-psum fills (2 tiles x 2 levels each)
    NCHUNK = 4           # 512-col chunks of the [16, 2048] output psum
    NWAVE = 8            # Z waves (4 (l,o) pairs each)

    with tile.TileContext(nc) as tc:
        with (
            tc.tile_pool(name="const", bufs=1) as cpool,
            tc.tile_pool(name="bits", bufs=1) as bpool,
            tc.tile_pool(name="codes", bufs=1) as cdpool,
            tc.tile_pool(name="eq", bufs=3) as eqpool,
            tc.tile_pool(name="u", bufs=1) as upool,
            tc.tile_pool(name="out", bufs=1) as opool,
            tc.tile_pool(name="psum_code", bufs=2,
                         space=bass.MemorySpace.PSUM) as pc,
            tc.tile_pool(name="psum_z", bufs=2,
                         space=bass.MemorySpace.PSUM) as pz,
            tc.tile_pool(name="psum_y", bufs=4,
                         space=bass.MemorySpace.PSUM) as py,
        ):
            xt_sb = cpool.tile([IN, B], F32, tag="xt")
            c_sb = cpool.tile([128, 2], F32, tag="consts")
            onesq_sb = cpool.tile([128, OL], F16, tag="onesq")
            qres_sb = cpool.tile([128, LEVELS * NV * OL], F16, tag="qres")
            nc.sync.dma_start(xt_sb[:], xt[:])
            nc.sync.dma_start(c_sb[:], consts[:])
            # g in 4 chunks so code matmuls start before the full DMA lands
            g_sb = []
            for q in range(4):
                gq = cpool.tile([IN, 4 * 128], F16, tag=f"g{q}",
                                name=f"g_sb{q}")
                nc.sync.dma_start(gq[:], g[:, q * 512:(q + 1) * 512])
                g_sb.append(gq)
            # M matrices [p, (l, o, q)] in 4 chunks
            mq_sb = cpool.tile([128, LEVELS * OL * 128], F16, tag="mq")
            MCH = LEVELS * OL * 128 // 4
            for q in range(4):
                nc.sync.dma_start(mq_sb[:, q * MCH:(q + 1) * MCH],
                                  mq[:, q * MCH:(q + 1) * MCH])
            nc.sync.dma_start(qres_sb[:], qres[:])
            nc.gpsimd.memset(onesq_sb[:], 1.0)

            # ---- sign bits (fp16 0/1, j on partitions), both levels in one
            # [128, 256] tensor so each code matmul streams 256 cols ----
            bits_cat = bpool.tile([IN, LEVELS * B], F16, tag="bits")
            bit1 = bits_cat[:, 0:B]
            bit2 = bits_cat[:, B:2 * B]
            nc.vector.tensor_scalar(bit1, xt_sb[:], 0.0, None,
                                    mybir.AluOpType.is_ge)
            rc = bpool.tile([IN, B], F32, tag="rc")
            nc.vector.scalar_tensor_tensor(rc[:], bit1, c_sb[:, 0:1],
                                           xt_sb[:], mybir.AluOpType.mult,
                                           mybir.AluOpType.add)
            nc.vector.tensor_scalar(bit2, rc[:], c_sb[:, 1:2], None,
                                    mybir.AluOpType.is_ge)
            # replicated bits for the qmult waves (4 o's per wave)
            brep = bpool.tile([IN, LEVELS * 512], F16, tag="brep")
            for l in range(LEVELS):
                for r in range(4):
                    nc.vector.tensor_copy(
                        brep[:, l * 512 + r * 128:l * 512 + r * 128 + 128],
                        bits_cat[:, l * B:(l + 1) * B])

            # ---- code matmuls + PSUM->SBUF drains (ACT/GPSIMD split) ----
            # codes_cat[j, (l, tile, b)]: both levels in one fp16 tensor
            codes = cdpool.tile([128, LEVELS * NTILE * B], F16, tag="codes")
            for f in range(NFILL):
                cps = pc.tile([128, 512], F32, tag="codepsum",
                              name=f"cps{f}")
                for dt_i in range(2):
                    t_i = 2 * f + dt_i
                    gq = g_sb[t_i // 4]
                    gcol = (t_i % 4) * 128
                    # one matmul streams both levels (256 cols)
                    nc.tensor.matmul(
                        cps[:].rearrange("p (d l b) -> p d (l b)", d=2,
                                         l=2)[:, dt_i],
                        gq[:, gcol:gcol + 128],
                        bits_cat[:],
                        start=True, stop=True,
                    )
                # drain: src (d, l, b) -> codes[(l, t, b)] at tiles 2f, 2f+1
                src = cps[:].rearrange("p (d l b) -> p l d b", d=2, l=2)
                dst = codes[:].rearrange("p (l t b) -> p l t b", l=2,
                                         t=16)[:, :, 2 * f:2 * f + 2, :]
                nc.scalar.copy(dst, src)

            y_ps = [py.tile([OL, 512], F32, tag="ypsum", name=f"yps{c}")
                    for c in range(NCHUNK)]

            # ---- quadratic forms: Z = M^T @ bits per (l, o) ----
            zps = []
            for w in range(NWAVE):
                zw = pz.tile([128, 512], F32, tag="zpsum", name=f"z{w}")
                zps.append(zw)
                l = w // 4
                for zo in range(4):
                    o = (w % 4) * 4 + zo
                    mcol = (l * OL + o) * 128
                    nc.tensor.matmul(
                        zw[:, zo * 128:(zo + 1) * 128],
                        mq_sb[:, mcol:mcol + 128],
                        bits_cat[:, l * B:(l + 1) * B],
                        start=True, stop=True,
                    )

            # ---- one-hot residual planes + streamed contraction;
            # qmult waves interleaved so DVE frees Z psum bufs steadily ----
            u_sb = upool.tile([128, LEVELS * OL * B], F16, tag="u")

            def emit_qmult(w):
                l = w // 4
                nc.vector.tensor_tensor(
                    u_sb[:, w * 512:(w + 1) * 512],
                    zps[w][:],
                    brep[:, l * 512:(l + 1) * 512],
                    mybir.AluOpType.mult)

            for vi, v in enumerate(VRES):
                eq = eqpool.tile([128, LEVELS * NTILE * B], F16, tag="eq")
                nc.vector.tensor_scalar(eq[:], codes[:], float(v), None,
                                        mybir.AluOpType.is_equal)
                for l in range(LEVELS):
                    qcol = (l * NV + vi) * OL
                    for c in range(NCHUNK):
                        nc.tensor.matmul(
                            y_ps[c][:],
                            qres_sb[:, qcol:qcol + OL],
                            eq[:, l * 2048 + c * 512:l * 2048 + c * 512 + 512],
                            start=(vi == 0 and l == 0), stop=False,
                        )
                if 2 * vi < NWAVE:
                    emit_qmult(2 * vi)
                if 2 * vi + 1 < NWAVE:
                    emit_qmult(2 * vi + 1)

            # ---- u-sum: every output row gets the column sum of u ----
            for l in range(LEVELS):
                for c in range(NCHUNK):
                    nc.tensor.matmul(
                        y_ps[c][:],
                        onesq_sb[:],
                        u_sb[:, l * 2048 + c * 512:l * 2048 + c * 512 + 512],
                        start=False, stop=(l == LEVELS - 1),
                    )

            y_sb = opool.tile([OL, NTILE * B], F16, tag="ysb")
            for c in range(NCHUNK):
                nc.scalar.copy(y_sb[:, c * 512:(c + 1) * 512], y_ps[c][:])
            nc.sync.dma_start(y[:], y_sb[:])

    nc.compile()
    return nc


def _host_prep(x, weight, bias, means):
    """Weight-static preprocessing: per-level LUTs Q_l[t, v] (fp64)."""
    w = weight.astype(np.float64)
    m = np.abs(means.astype(np.float64))
    cc = np.arange(KK)
    tt = (2 * ((cc[:, None] >> np.arange(K)[None, :]) & 1) - 1).astype(
        np.float64)          # [c, i]
    sig = tt                  # same construction for sign patterns [v, i]

    qs = []
    for l in range(LEVELS):
        # M[v, c] = prod_i (1 + m_l * sig[v,i] * tt[c,i]) / 2
        M = np.prod((1.0 + m[l] * sig[:, None, :] * tt[None, :, :]) * 0.5,
                    axis=-1)  # [v, c]
        q = w @ M.T           # [T, KK]
        qs.append(q)
    return qs


def _feat(codes):
    """11-dim feature vector [1, b0..b3, pair products] per code."""
    codes = np.asarray(codes)
    b = ((codes[:, None] >> np.arange(K)[None, :]) & 1).astype(np.float64)
    cols = [np.ones(len(codes))] + [b[:, i] for i in range(K)]
    cols += [b[:, i] * b[:, k] for (i, k) in PAIRS]
    return np.stack(cols, axis=1)  # [n, 11]


def _quad_fit(qs):
    """Fit alpha/gamma/pair coefs exactly at C11; residual at VRES.

    Returns coef[l][T, 11] and resid[l][T, NV]."""
    A = _feat(C11)                 # [11, 11]
    Ainv = np.linalg.inv(A)
    Fres = _feat(VRES)             # [NV, 11]
    coefs, resid = [], []
    for l in range(LEVELS):
        c = qs[l][:, C11] @ Ainv.T          # [T, 11]
        r = qs[l][:, VRES] - c @ Fres.T     # [T, NV]
        coefs.append(c)
        resid.append(r)
    return coefs, resid


def _build_g(input_mask):
    G = np.zeros((IN, T), np.float64)
    cols = np.repeat(np.arange(T), K)
    vals = np.tile(2.0 ** np.arange(K), T)
    np.add.at(G, (input_mask.astype(np.int64), cols), vals)
    return G


def _build_m(input_mask, coefs, t0):
    """M[p, (l, o, q)] for this core's OL out-features starting at table t0.

    Quadratic-form matrices: diagonal gets gamma_i at idx_i; entry
    (idx_i, idx_k) accumulates the pair coefficient (single-sided; the
    device computes bits^T M bits so diagonal collisions are absorbed by
    bit^2 = bit)."""
    idx = input_mask.astype(np.int64).reshape(T, K)[t0:t0 + T_C]  # [T_C, 4]
    Ms = np.zeros((LEVELS, OL, 128, 128), np.float64)
    tloc = np.arange(T_C)
    o_of_t = tloc // IN
    for l in range(LEVELS):
        cf = coefs[l][t0:t0 + T_C]  # [T_C, 11]
        for i in range(K):
            np.add.at(Ms, (l, o_of_t, idx[:, i], idx[:, i]), cf[:, 1 + i])
        for pi, (i, k) in enumerate(PAIRS):
            np.add.at(Ms, (l, o_of_t, idx[:, i], idx[:, k]), cf[:, 5 + pi])
    # -> [p, (l, o, q)]
    return Ms.transpose(2, 0, 1, 3).reshape(128, -1)


def _make_in_maps(x, weight, bias, means, input_mask):
    qs = _host_prep(x, weight, bias, means)
    coefs, resid = _quad_fit(qs)
    G = _build_g(input_mask)

    m0 = float(np.abs(means.astype(np.float64))[0])
    consts = np.zeros((128, 2), np.float32)
    consts[:, 0] = -2.0 * m0
    consts[:, 1] = -m0
    xt = np.ascontiguousarray(x.astype(np.float32).T)

    # cvec[o] = bias[o] + sum_l sum_j alpha_l[o*IN+j]
    cvec_full = bias.astype(np.float64).copy()
    for l in range(LEVELS):
        cvec_full += coefs[l][:, 0].reshape(OUT, IN).sum(-1)

    in_maps = []
    for cid in range(NCORES):
        t0 = cid * T_C
        gc = G[:, t0:t0 + T_C].astype(np.float16)
        mc = _build_m(input_mask, coefs, t0).astype(np.float16)
        # qres[j, (l, vi, o)] = resid_l[(o*IN+j), vi]
        qres_c = np.empty((128, LEVELS, NV, OL), np.float64)
        for l in range(LEVELS):
            rc = resid[l][t0:t0 + T_C].reshape(OL, IN, NV)
            qres_c[:, l, :, :] = rc.transpose(1, 2, 0)
        in_maps.append({
            "xt": xt,
            "consts": consts,
            "g": np.ascontiguousarray(gc),
            "mq": np.ascontiguousarray(mc),
            "qres": np.ascontiguousarray(
                qres_c.reshape(128, -1).astype(np.float16)),
        })
    return in_maps, cvec_full


def kernel(x, weight, bias, means, input_mask):
    global _CACHED_NC
    if _CACHED_NC is None:
        _CACHED_NC = _build_nc()
    nc = _CACHED_NC

    in_maps, cvec_full = _make_in_maps(x, weight, bias, means, input_mask)
    res = run_bass_kernel_spmd(nc, in_maps, list(range(NCORES)))
    globals()["_LAST_RESULTS"] = res
    # y[o_local, tile*128 + b]: diagonal tile == o_local holds the result;
    # the per-o constant (bias + LUT fit constants) is added host-side
    out = np.empty((B, OUT), np.float32)
    for cid in range(NCORES):
        yc = res.results[cid]["y"].astype(np.float32)  # [OL, 2048]
        for o in range(OL):
            out[:, cid * OL + o] = yc[o, o * 128:(o + 1) * 128]
    out += cvec_full.astype(np.float32)[None, :]
    return out
